# revision 32
# baseline (speedup 1.0000x reference)
"""NeRF-small MLP Bass kernel for Trainium2, 8-core data parallel.

v2 layout: hidden-on-partitions, points-on-free-dim, with HOST-side
input/output permutation so the device does zero transposes and zero
repacks.

Input: host pre-packs x into t4-layout [N_ST, 128, 512] bf16 where row
8q+c = channel c (c<6, pads zero) of slab q, col 128k+j = point
base + 2048k + 16j + q of the super-tile.  One contiguous HWDGE DMA per
super-tile.

Compute per pair of slabs j (1024 pts; every matmul K<=128, N=512):
  H0 = w0big_j x t4                     (3->64 both slabs)
  H1 = s1big x h0                       (64->64)
  CV = c0vbig_j x t4 + compbig x h1     (view path + composed geo path)
  C1 = c1big x c0
  C2 = c2big x c1h
  C3C += c3big_W x c2h ; += sigbig_W x h1   (final layer)

Output: one [128,512] f32 PSUM bank accumulates the final layer for 16
pairs (2 super-tiles): colors of pair W in rows 6W..6W+5, sigma_raw in
rows 96+2W..97+2W (other stationary columns are zero -> +0).  Evacuated
once per 2 super-tiles with ONE ACT Copy [128,512] into an SBUF tile
that is stored channel-major bf16 (colors + raw sigma).  Sigma then
takes a side path that keeps ACT/DVE free: a gpsimd SBUF-shuffle DMA
re-shapes the raw [32,512] sigma block to [128,128] so the softplus
polynomial (sigma = relu(x) + t*R(t), t = exp(-|x|), R deg-3) runs as
dense [128,128] ops: Abs/Exp on ACT, the 6-op Horner + relu + add on
the otherwise-idle Pool engine.  The bf16 result is stored as its own
small tensor; the host unpermutes everything to [N,4] f32.  No device
transposes, one activation table (Relu/Copy/Abs/Exp).

PSUM banks (8): H0 x2, H1, CV, C1, C2, C3C x2.
Evacuation engines statically balanced: ACT {h0, c0, c2h 5/8, output
Copy/Abs/Exp}, DVE {h1, c1h, c2h 3/8}.
"""

import numpy as np
import ml_dtypes

N_TOTAL = 1048576
N_CORES = 8
NPC = N_TOTAL // N_CORES       # 131072 points per core
ST = 8192                      # points per super-tile (4 chunks of 2048)
N_ST = NPC // ST               # 16
FP8_SCALE = 1024.0             # power-of-2, exact to un-scale

_CACHE = {}


def _pack_weights(ws0, ws1, ws2, wc0, wc1, wc2, wc3):
    """Build block-diagonal 'big' stationary matrices."""
    bf16 = ml_dtypes.bfloat16
    f32 = np.float32
    ws0, ws1, ws2, wc0, wc1, wc2, wc3 = [
        np.asarray(w, f32) for w in (ws0, ws1, ws2, wc0, wc1, wc2, wc3)
    ]
    w0big = np.zeros((128, 8 * 128), f32)
    c0vbig = np.zeros((128, 8 * 128), f32)
    for j in range(8):
        for q, off in ((2 * j, 0), (2 * j + 1, 64)):
            w0big[8 * q: 8 * q + 3, 128 * j + off: 128 * j + off + 64] = ws0
            c0vbig[8 * q + 3: 8 * q + 6, 128 * j + off: 128 * j + off + 64] = wc0[0:3]
    s1big = np.zeros((128, 128), f32)
    s1big[0:64, 0:64] = ws1
    s1big[64:128, 64:128] = ws1
    # geo path composed offline: geo @ wc0[3:18] = h1 @ (ws2[:,1:16] @ wc0[3:18])
    comp = ws2[:, 1:16] @ wc0[3:18]
    compbig = np.zeros((128, 128), f32)
    compbig[0:64, 0:64] = comp
    compbig[64:128, 64:128] = comp
    c1big = np.zeros((128, 128), f32)
    c1big[0:64, 0:64] = wc1
    c1big[64:128, 64:128] = wc1
    c2big = np.zeros((128, 128), f32)
    c2big[0:64, 0:64] = wc2
    c2big[64:128, 64:128] = wc2
    # Final layer: 16 pair-variants W, each a stationary writing the shared
    # [128,512] C3C accumulation bank.
    c3big = np.zeros((128, 16 * 128), f32)
    sigbig = np.zeros((128, 16 * 128), f32)
    for W in range(16):
        c3big[0:64, 128 * W + 6 * W: 128 * W + 6 * W + 3] = wc3[:, 0:3]
        c3big[64:128, 128 * W + 6 * W + 3: 128 * W + 6 * W + 6] = wc3[:, 0:3]
        sigbig[0:64, 128 * W + 96 + 2 * W] = ws2[:, 0]
        sigbig[64:128, 128 * W + 97 + 2 * W] = ws2[:, 0]

    return {
        "w0big": w0big.astype(bf16), "c0vbig": c0vbig.astype(bf16),
        "s1big": s1big.astype(bf16), "compbig": compbig.astype(bf16),
        "c1big": c1big.astype(bf16), "c2big": c2big.astype(bf16),
        "c3big": c3big.astype(bf16), "sigbig": sigbig.astype(bf16),
    }


def _pack_input(x):
    """[N,6] f32 -> [cores, N_ST, 128, 512] bf16 t4 layout:
    t4[8q+c, 128k+j] = x[base + 2048k + 16j + q, c], pads (c=6,7) zero."""
    bf16 = ml_dtypes.bfloat16
    x = np.asarray(x, np.float32).reshape(N_CORES, N_ST, 4, 128, 16, 6)
    t4 = np.zeros((N_CORES, N_ST, 16, 8, 4, 128), bf16)
    # [core, s, k, j, q, c] -> [core, s, q, c, k, j]
    t4[:, :, :, 0:6] = x.transpose(0, 1, 4, 5, 2, 3).astype(bf16)
    return t4.reshape(N_CORES, N_ST, 128, 512)


def _unpack_output(dev, dev2):
    """dev [cores, N_ST//2, 128, 512] (colors rows 0-95), dev2 [cores,
    N_ST//2, 128, 128] (softplus'd sigma, shuffled) -> [N, 4] f32.
    Pair W (0..15) within group g spans ST = 2g + W//8, slabs q = 2w, 2w+1
    (w = W%8); its column 128k+j is point (ST*4+k)*2048 + 16j + q.
    dev2[32k + 2W + par, j] = sigma of that point."""
    ns2 = N_ST // 2
    dev = dev.reshape(N_CORES, ns2, 128, 4, 128)
    # colors rows 6W+3*par+c
    colors = dev[:, :, :96].reshape(N_CORES, ns2, 2, 8, 2, 3, 4, 128)
    # -> [core, g, st2, k, j, w, par, c]
    colors = colors.transpose(0, 1, 2, 6, 7, 3, 4, 5)
    colors = np.ascontiguousarray(colors).reshape(N_TOTAL, 3).astype(np.float32)
    sigma = dev2.reshape(N_CORES, ns2, 4, 2, 8, 2, 128)  # [core,g,k,st2,w,par,j]
    sigma = sigma.transpose(0, 1, 3, 2, 6, 4, 5)         # [core,g,st2,k,j,w,par]
    sigma = np.ascontiguousarray(sigma).reshape(N_TOTAL, 1).astype(np.float32)
    return np.concatenate([colors, sigma], axis=1)


def _build(npts):
    import concourse.mybir as mybir
    from concourse import bacc, tile
    from concourse.bass import AP

    dt = mybir.dt
    f32, bf16 = dt.float32, dt.bfloat16
    AF = mybir.ActivationFunctionType

    ALU = mybir.AluOpType
    # ln(1+t)/t ~= B0 + B1 t + B2 t^2 + B3 t^3 on [0,1] (Chebyshev fit,
    # max abs err in t*R(t) is 5.1e-4)
    B0, B1, B2, B3 = 0.99930126, -0.48463524, 0.25187429, -0.0738988

    n_st = npts // ST
    nc = bacc.Bacc()
    x8t = nc.dram_tensor("x8t", [n_st, 128, 512], bf16, kind="ExternalInput")
    out = nc.dram_tensor("out", [n_st // 2, 128, 512], bf16, kind="ExternalOutput")
    out2 = nc.dram_tensor("out2", [n_st // 2, 128, 128], bf16, kind="ExternalOutput")
    wshapes = {
        "w0big": [128, 8 * 128], "c0vbig": [128, 8 * 128],
        "s1big": [128, 128], "compbig": [128, 128],
        "c1big": [128, 128], "c2big": [128, 128],
        "c3big": [128, 16 * 128], "sigbig": [128, 16 * 128],
    }
    wdr = {n: nc.dram_tensor(n, s, bf16, kind="ExternalInput")
           for n, s in wshapes.items()}

    with tile.TileContext(nc) as tc:
        with (
            tc.tile_pool(name="const", bufs=1) as constp,
            tc.tile_pool(name="t4", bufs=4) as t4p,
            tc.tile_pool(name="act", bufs=4) as actp,
            tc.tile_pool(name="ost", bufs=2) as ostp,
            tc.tile_pool(name="psum", bufs=1, space="PSUM") as psump,
            tc.tile_pool(name="psc2", bufs=1, space="PSUM") as psc2p,
            tc.tile_pool(name="psh0", bufs=1, space="PSUM") as psh0p,
            tc.tile_pool(name="psc3", bufs=2, space="PSUM") as psc3p,
        ):
            W = {}
            for name, shp in wshapes.items():
                t = constp.tile(shp, bf16, tag=name)
                nc.gpsimd.dma_start(t[:], wdr[name][:])
                W[name] = t

            def prefetch(s):
                t4 = t4p.tile([128, 512], bf16, tag="t4")
                with tc.high_priority():
                    nc.sync.dma_start(t4[:], x8t[s])
                return t4

            t4q = [prefetch(0), prefetch(1), prefetch(2)]

            C3C = None
            sig_pend = None   # (grp, sg) awaiting the softplus chain

            def emit_c3(Widx, grp, c2h, h1):
                nonlocal C3C, sig_pend
                if Widx == 0:
                    C3C = psc3p.tile([128, 512], f32, tag="C3C")
                nc.tensor.matmul(C3C[:], W["c3big"][:, 128 * Widx: 128 * (Widx + 1)],
                                 c2h, start=(Widx == 0), stop=False)
                nc.tensor.matmul(C3C[:], W["sigbig"][:, 128 * Widx: 128 * (Widx + 1)],
                                 h1[:], start=False, stop=(Widx == 15))
                if Widx == 15:
                    # one evac releases the bank: colors + raw sigma
                    ost = ostp.tile([128, 512], bf16, tag="ost")
                    nc.vector.tensor_copy(ost[:], C3C[:])
                    nc.sync.dma_start(out[grp], ost[:])
                    # shuffle raw sigma [32,512] -> [128,128] for the poly
                    sg = ostp.tile([128, 128], bf16, tag="sg")
                    for k in range(4):
                        nc.sync.dma_start(
                            sg[32 * k: 32 * (k + 1), :],
                            ost[96:128, 128 * k: 128 * (k + 1)])
                    sig_pend = (grp, sg)

            def sig_chain(grp, sg, last=False):
                # softplus(x) = relu(x) + t*R(t), t = exp(-|x|); dense
                # [128,128] tiles: Abs/Exp on ACT, the rest on Pool (steady
                # state) or DVE (final group, latency-critical tail)
                eng = nc.vector if last else nc.gpsimd
                spa = ostp.tile([128, 128], f32, tag="spa")
                nc.scalar.activation(spa[:], sg[:], AF.Abs)
                spt = ostp.tile([128, 128], f32, tag="spt")
                nc.scalar.activation(spt[:], spa[:], AF.Exp, scale=-1.0)
                t2 = ostp.tile([128, 128], f32, tag="t2")
                eng.tensor_tensor(t2[:], spt[:], spt[:], op=ALU.mult)
                pA = ostp.tile([128, 128], f32, tag="pA")
                eng.tensor_scalar(pA[:], spt[:], B1, B0, op0=ALU.mult, op1=ALU.add)
                pB = ostp.tile([128, 128], f32, tag="pB")
                eng.tensor_scalar(pB[:], spt[:], B3, B2, op0=ALU.mult, op1=ALU.add)
                pC = ostp.tile([128, 128], f32, tag="pC")
                eng.tensor_tensor(pC[:], pB[:], t2[:], op=ALU.mult)
                pD = ostp.tile([128, 128], f32, tag="pD")
                eng.tensor_tensor(pD[:], pA[:], pC[:], op=ALU.add)
                pP = ostp.tile([128, 128], f32, tag="pP")
                eng.tensor_tensor(pP[:], pD[:], spt[:], op=ALU.mult)
                pr = ostp.tile([128, 128], f32, tag="pr")
                eng.tensor_scalar_max(pr[:], sg[:], 0.0)
                sg2 = ostp.tile([128, 128], bf16, tag="sg2")
                eng.tensor_tensor(sg2[:], pP[:], pr[:], op=ALU.add)
                nc.sync.dma_start(out2[grp], sg2[:])

            defq = []       # pending C3/SIG emissions (c2h evac'd, order kept)
            staged = None   # entries staged one body to let the fused evac land
            C2T = None
            c2q = []        # (Widx, grp, h1) awaiting fused evac
            for s in range(n_st):
                t4 = t4q.pop(0)
                if s + 3 < n_st:
                    t4q.append(prefetch(s + 3))

                for j in range(8):
                    Widx = 8 * (s % 2) + j
                    if staged is not None:
                        defq.extend(staged)
                        staged = None
                    if j == 2 and sig_pend is not None:
                        sig_chain(*sig_pend)
                        sig_pend = None
                    H0 = psh0p.tile([128, 512], f32, tag="H0")
                    nc.tensor.matmul(H0[:], W["w0big"][:, 128 * j: 128 * (j + 1)],
                                     t4[:], start=True, stop=True)
                    h0 = actp.tile([128, 512], bf16, tag="h0")
                    nc.scalar.activation(h0[:], H0[:], AF.Relu)

                    H1 = psump.tile([128, 512], f32, tag="H1")
                    nc.tensor.matmul(H1[:], W["s1big"][:], h0[:], start=True, stop=True)
                    h1 = actp.tile([128, 512], bf16, tag="h1")
                    nc.vector.tensor_scalar_max(h1[:], H1[:], 0.0)

                    CV = psump.tile([128, 512], f32, tag="CV")
                    nc.tensor.matmul(CV[:], W["c0vbig"][:, 128 * j: 128 * (j + 1)],
                                     t4[:], start=True, stop=False)
                    nc.tensor.matmul(CV[:], W["compbig"][:], h1[:], start=False, stop=True)
                    c0 = actp.tile([128, 512], bf16, tag="c0")
                    nc.scalar.activation(c0[:], CV[:], AF.Relu)

                    C1 = psump.tile([128, 512], f32, tag="C1")
                    nc.tensor.matmul(C1[:], W["c1big"][:], c0[:], start=True, stop=True)
                    c1h = actp.tile([128, 512], bf16, tag="c1h")
                    if j % 2 == 0:
                        nc.scalar.activation(c1h[:], C1[:], AF.Relu)
                    else:
                        nc.vector.tensor_scalar_max(c1h[:], C1[:], 0.0)

                    # C2 of pairs (2t, 2t+1) share a [128,1024] 2-bank tile;
                    # ONE fused evac after the odd pair.  Its consumers (C3/
                    # SIG) are deferred anyway, so the longer fused-evac
                    # latency is off the critical path.
                    if j % 2 == 0:
                        C2T = psc2p.tile([128, 1024], f32, tag="C2T")
                    half = slice(512 * (j % 2), 512 * (j % 2) + 512)
                    nc.tensor.matmul(C2T[:, half], W["c2big"][:], c1h[:],
                                     start=True, stop=True)
                    c2q.append((Widx, s // 2, h1))
                    if j % 2 == 1:
                        hc = actp.tile([128, 1024], bf16, tag="hc")
                        nc.vector.tensor_scalar_max(hc[:], C2T[:], 0.0)
                        staged = [(Wi, gi, hc[:, 512 * i: 512 * i + 512], h1i)
                                  for i, (Wi, gi, h1i) in enumerate(c2q)]
                        c2q = []

                    # emit one deferred C3/SIG per pair body
                    if defq:
                        emit_c3(*defq.pop(0))
            if staged is not None:
                defq.extend(staged)
            while defq:
                emit_c3(*defq.pop(0))
            if sig_pend is not None:
                sig_chain(*sig_pend, last=True)
    nc.compile()
    return nc


def _run(inputs, npts=NPC, trace=False, cores=N_CORES):
    from concourse import bass_utils

    key = npts
    if key not in _CACHE:
        _CACHE[key] = _build(npts)
    nc = _CACHE[key]
    wm = _pack_weights(inputs["ws0"], inputs["ws1"], inputs["ws2"],
                       inputs["wc0"], inputs["wc1"], inputs["wc2"], inputs["wc3"])
    t4 = _pack_input(inputs["x"])
    in_maps = [dict(wm, x8t=np.ascontiguousarray(t4[c])) for c in range(cores)]
    res = bass_utils.run_bass_kernel_spmd(
        nc, in_maps, core_ids=list(range(cores)), trace=trace)
    dev = np.stack([r["out"] for r in res.results], axis=0)
    dev2 = np.stack([r["out2"] for r in res.results], axis=0)
    return _unpack_output(dev, dev2), res


def kernel(**inputs):
    out, _ = _run(inputs)
    return out.astype(np.float32)


# revision 33
# speedup vs baseline: 1.0016x; 1.0016x over previous
"""NeRF-small MLP Bass kernel for Trainium2, 8-core data parallel.

v2 layout: hidden-on-partitions, points-on-free-dim, with HOST-side
input/output permutation so the device does zero transposes and zero
repacks.

Input: host pre-packs x into t4-layout [N_ST, 128, 512] bf16 where row
8q+c = channel c (c<6, pads zero) of slab q, col 128k+j = point
base + 2048k + 16j + q of the super-tile.  One contiguous HWDGE DMA per
super-tile.

Compute per pair of slabs j (1024 pts; every matmul K<=128, N=512):
  H0 = w0big_j x t4                     (3->64 both slabs)
  H1 = s1big x h0                       (64->64)
  CV = c0vbig_j x t4 + compbig x h1     (view path + composed geo path)
  C1 = c1big x c0
  C2 = c2big x c1h
  C3C += c3big_W x c2h ; += sigbig_W x h1   (final layer)

Output: one [128,512] f32 PSUM bank accumulates the final layer for 16
pairs (2 super-tiles): colors of pair W in rows 6W..6W+5, sigma_raw in
rows 96+2W..97+2W (other stationary columns are zero -> +0).  Evacuated
once per 2 super-tiles with ONE ACT Copy [128,512] into an SBUF tile
that is stored channel-major bf16 (colors + raw sigma).  Sigma then
takes a side path that keeps ACT/DVE free: a gpsimd SBUF-shuffle DMA
re-shapes the raw [32,512] sigma block to [128,128] so the softplus
polynomial (sigma = relu(x) + t*R(t), t = exp(-|x|), R deg-3) runs as
dense [128,128] ops: Abs/Exp on ACT, the 6-op Horner + relu + add on
the otherwise-idle Pool engine.  The bf16 result is stored as its own
small tensor; the host unpermutes everything to [N,4] f32.  No device
transposes, one activation table (Relu/Copy/Abs/Exp).

PSUM banks (8): H0 x2, H1, CV, C1, C2, C3C x2.
Evacuation engines statically balanced: ACT {h0, c0, c2h 5/8, output
Copy/Abs/Exp}, DVE {h1, c1h, c2h 3/8}.
"""

import numpy as np
import ml_dtypes

N_TOTAL = 1048576
N_CORES = 8
NPC = N_TOTAL // N_CORES       # 131072 points per core
ST = 8192                      # points per super-tile (4 chunks of 2048)
N_ST = NPC // ST               # 16
FP8_SCALE = 1024.0             # power-of-2, exact to un-scale

_CACHE = {}


def _pack_weights(ws0, ws1, ws2, wc0, wc1, wc2, wc3):
    """Build block-diagonal 'big' stationary matrices."""
    bf16 = ml_dtypes.bfloat16
    f32 = np.float32
    ws0, ws1, ws2, wc0, wc1, wc2, wc3 = [
        np.asarray(w, f32) for w in (ws0, ws1, ws2, wc0, wc1, wc2, wc3)
    ]
    w0big = np.zeros((128, 8 * 128), f32)
    c0vbig = np.zeros((128, 8 * 128), f32)
    for j in range(8):
        for q, off in ((2 * j, 0), (2 * j + 1, 64)):
            w0big[8 * q: 8 * q + 3, 128 * j + off: 128 * j + off + 64] = ws0
            c0vbig[8 * q + 3: 8 * q + 6, 128 * j + off: 128 * j + off + 64] = wc0[0:3]
    s1big = np.zeros((128, 128), f32)
    s1big[0:64, 0:64] = ws1
    s1big[64:128, 64:128] = ws1
    # geo path composed offline: geo @ wc0[3:18] = h1 @ (ws2[:,1:16] @ wc0[3:18])
    comp = ws2[:, 1:16] @ wc0[3:18]
    compbig = np.zeros((128, 128), f32)
    compbig[0:64, 0:64] = comp
    compbig[64:128, 64:128] = comp
    c1big = np.zeros((128, 128), f32)
    c1big[0:64, 0:64] = wc1
    c1big[64:128, 64:128] = wc1
    c2big = np.zeros((128, 128), f32)
    c2big[0:64, 0:64] = wc2
    c2big[64:128, 64:128] = wc2
    # Final layer: 16 pair-variants W, each a stationary writing the shared
    # [128,512] C3C accumulation bank.
    c3big = np.zeros((128, 16 * 128), f32)
    sigbig = np.zeros((128, 16 * 128), f32)
    for W in range(16):
        c3big[0:64, 128 * W + 6 * W: 128 * W + 6 * W + 3] = wc3[:, 0:3]
        c3big[64:128, 128 * W + 6 * W + 3: 128 * W + 6 * W + 6] = wc3[:, 0:3]
        sigbig[0:64, 128 * W + 96 + 2 * W] = ws2[:, 0]
        sigbig[64:128, 128 * W + 97 + 2 * W] = ws2[:, 0]

    return {
        "w0big": w0big.astype(bf16), "c0vbig": c0vbig.astype(bf16),
        "s1big": s1big.astype(bf16), "compbig": compbig.astype(bf16),
        "c1big": c1big.astype(bf16), "c2big": c2big.astype(bf16),
        "c3big": c3big.astype(bf16), "sigbig": sigbig.astype(bf16),
    }


def _pack_input(x):
    """[N,6] f32 -> [cores, N_ST, 128, 512] bf16 t4 layout:
    t4[8q+c, 128k+j] = x[base + 2048k + 16j + q, c], pads (c=6,7) zero."""
    bf16 = ml_dtypes.bfloat16
    x = np.asarray(x, np.float32).reshape(N_CORES, N_ST, 4, 128, 16, 6)
    t4 = np.zeros((N_CORES, N_ST, 16, 8, 4, 128), bf16)
    # [core, s, k, j, q, c] -> [core, s, q, c, k, j]
    t4[:, :, :, 0:6] = x.transpose(0, 1, 4, 5, 2, 3).astype(bf16)
    return t4.reshape(N_CORES, N_ST, 128, 512)


def _unpack_output(dev, dev2):
    """dev [cores, N_ST//2, 128, 512] (colors rows 0-95), dev2 [cores,
    N_ST//2, 128, 128] (softplus'd sigma, shuffled) -> [N, 4] f32.
    Pair W (0..15) within group g spans ST = 2g + W//8, slabs q = 2w, 2w+1
    (w = W%8); its column 128k+j is point (ST*4+k)*2048 + 16j + q.
    dev2[32k + 2W + par, j] = sigma of that point."""
    ns2 = N_ST // 2
    dev = dev.reshape(N_CORES, ns2, 128, 4, 128)
    # colors rows 6W+3*par+c
    colors = dev[:, :, :96].reshape(N_CORES, ns2, 2, 8, 2, 3, 4, 128)
    # -> [core, g, st2, k, j, w, par, c]
    colors = colors.transpose(0, 1, 2, 6, 7, 3, 4, 5)
    colors = np.ascontiguousarray(colors).reshape(N_TOTAL, 3).astype(np.float32)
    sigma = dev2.reshape(N_CORES, ns2, 4, 2, 8, 2, 128)  # [core,g,k,st2,w,par,j]
    sigma = sigma.transpose(0, 1, 3, 2, 6, 4, 5)         # [core,g,st2,k,j,w,par]
    sigma = np.ascontiguousarray(sigma).reshape(N_TOTAL, 1).astype(np.float32)
    return np.concatenate([colors, sigma], axis=1)


def _build(npts):
    import concourse.mybir as mybir
    from concourse import bacc, tile
    from concourse.bass import AP

    dt = mybir.dt
    f32, bf16 = dt.float32, dt.bfloat16
    AF = mybir.ActivationFunctionType

    ALU = mybir.AluOpType
    # ln(1+t)/t ~= B0 + B1 t + B2 t^2 + B3 t^3 on [0,1] (Chebyshev fit,
    # max abs err in t*R(t) is 5.1e-4)
    B0, B1, B2, B3 = 0.99930126, -0.48463524, 0.25187429, -0.0738988

    n_st = npts // ST
    nc = bacc.Bacc()
    x8t = nc.dram_tensor("x8t", [n_st, 128, 512], bf16, kind="ExternalInput")
    out = nc.dram_tensor("out", [n_st // 2, 128, 512], bf16, kind="ExternalOutput")
    out2 = nc.dram_tensor("out2", [n_st // 2, 128, 128], bf16, kind="ExternalOutput")
    wshapes = {
        "w0big": [128, 8 * 128], "c0vbig": [128, 8 * 128],
        "s1big": [128, 128], "compbig": [128, 128],
        "c1big": [128, 128], "c2big": [128, 128],
        "c3big": [128, 16 * 128], "sigbig": [128, 16 * 128],
    }
    wdr = {n: nc.dram_tensor(n, s, bf16, kind="ExternalInput")
           for n, s in wshapes.items()}

    with tile.TileContext(nc) as tc:
        with (
            tc.tile_pool(name="const", bufs=1) as constp,
            tc.tile_pool(name="t4", bufs=4) as t4p,
            tc.tile_pool(name="act", bufs=4) as actp,
            tc.tile_pool(name="ost", bufs=2) as ostp,
            tc.tile_pool(name="psum", bufs=1, space="PSUM") as psump,
            tc.tile_pool(name="psc2", bufs=1, space="PSUM") as psc2p,
            tc.tile_pool(name="psh0", bufs=2, space="PSUM") as psh0p,
            tc.tile_pool(name="psc3", bufs=1, space="PSUM") as psc3p,
        ):
            W = {}
            for name, shp in wshapes.items():
                t = constp.tile(shp, bf16, tag=name)
                nc.gpsimd.dma_start(t[:], wdr[name][:])
                W[name] = t

            def prefetch(s):
                t4 = t4p.tile([128, 512], bf16, tag="t4")
                with tc.high_priority():
                    nc.sync.dma_start(t4[:], x8t[s])
                return t4

            t4q = [prefetch(0), prefetch(1), prefetch(2)]

            C3C = None
            sig_pend = None   # (grp, sg) awaiting the softplus chain

            def emit_c3(Widx, grp, c2h, h1):
                nonlocal C3C, sig_pend
                if Widx == 0:
                    C3C = psc3p.tile([128, 512], f32, tag="C3C")
                nc.tensor.matmul(C3C[:], W["c3big"][:, 128 * Widx: 128 * (Widx + 1)],
                                 c2h, start=(Widx == 0), stop=False)
                nc.tensor.matmul(C3C[:], W["sigbig"][:, 128 * Widx: 128 * (Widx + 1)],
                                 h1[:], start=False, stop=(Widx == 15))
                if Widx == 15:
                    # one evac releases the bank: colors + raw sigma
                    ost = ostp.tile([128, 512], bf16, tag="ost")
                    nc.vector.tensor_copy(ost[:], C3C[:])
                    nc.sync.dma_start(out[grp], ost[:])
                    # shuffle raw sigma [32,512] -> [128,128] for the poly
                    sg = ostp.tile([128, 128], bf16, tag="sg")
                    for k in range(4):
                        nc.sync.dma_start(
                            sg[32 * k: 32 * (k + 1), :],
                            ost[96:128, 128 * k: 128 * (k + 1)])
                    sig_pend = (grp, sg)

            def sig_chain(grp, sg, last=False):
                # softplus(x) = relu(x) + t*R(t), t = exp(-|x|); dense
                # [128,128] tiles: Abs/Exp on ACT, the rest on Pool (steady
                # state) or DVE (final group, latency-critical tail)
                eng = nc.vector if last else nc.gpsimd
                spa = ostp.tile([128, 128], f32, tag="spa")
                nc.scalar.activation(spa[:], sg[:], AF.Abs)
                spt = ostp.tile([128, 128], f32, tag="spt")
                nc.scalar.activation(spt[:], spa[:], AF.Exp, scale=-1.0)
                t2 = ostp.tile([128, 128], f32, tag="t2")
                eng.tensor_tensor(t2[:], spt[:], spt[:], op=ALU.mult)
                pA = ostp.tile([128, 128], f32, tag="pA")
                eng.tensor_scalar(pA[:], spt[:], B1, B0, op0=ALU.mult, op1=ALU.add)
                pB = ostp.tile([128, 128], f32, tag="pB")
                eng.tensor_scalar(pB[:], spt[:], B3, B2, op0=ALU.mult, op1=ALU.add)
                pC = ostp.tile([128, 128], f32, tag="pC")
                eng.tensor_tensor(pC[:], pB[:], t2[:], op=ALU.mult)
                pD = ostp.tile([128, 128], f32, tag="pD")
                eng.tensor_tensor(pD[:], pA[:], pC[:], op=ALU.add)
                pP = ostp.tile([128, 128], f32, tag="pP")
                eng.tensor_tensor(pP[:], pD[:], spt[:], op=ALU.mult)
                pr = ostp.tile([128, 128], f32, tag="pr")
                eng.tensor_scalar_max(pr[:], sg[:], 0.0)
                sg2 = ostp.tile([128, 128], bf16, tag="sg2")
                eng.tensor_tensor(sg2[:], pP[:], pr[:], op=ALU.add)
                nc.sync.dma_start(out2[grp], sg2[:])

            defq = []       # pending C3/SIG emissions (c2h evac'd, order kept)
            staged = None   # entries staged one body to let the fused evac land
            C2T = None
            c2q = []        # (Widx, grp, h1) awaiting fused evac
            for s in range(n_st):
                t4 = t4q.pop(0)
                if s + 3 < n_st:
                    t4q.append(prefetch(s + 3))

                for j in range(8):
                    Widx = 8 * (s % 2) + j
                    if staged is not None:
                        defq.extend(staged)
                        staged = None
                    if j == 2 and sig_pend is not None:
                        sig_chain(*sig_pend)
                        sig_pend = None
                    H0 = psh0p.tile([128, 512], f32, tag="H0")
                    nc.tensor.matmul(H0[:], W["w0big"][:, 128 * j: 128 * (j + 1)],
                                     t4[:], start=True, stop=True)
                    h0 = actp.tile([128, 512], bf16, tag="h0")
                    nc.scalar.activation(h0[:], H0[:], AF.Relu)

                    H1 = psump.tile([128, 512], f32, tag="H1")
                    nc.tensor.matmul(H1[:], W["s1big"][:], h0[:], start=True, stop=True)
                    h1 = actp.tile([128, 512], bf16, tag="h1")
                    nc.vector.tensor_scalar_max(h1[:], H1[:], 0.0)

                    CV = psump.tile([128, 512], f32, tag="CV")
                    nc.tensor.matmul(CV[:], W["c0vbig"][:, 128 * j: 128 * (j + 1)],
                                     t4[:], start=True, stop=False)
                    nc.tensor.matmul(CV[:], W["compbig"][:], h1[:], start=False, stop=True)
                    c0 = actp.tile([128, 512], bf16, tag="c0")
                    nc.scalar.activation(c0[:], CV[:], AF.Relu)

                    C1 = psump.tile([128, 512], f32, tag="C1")
                    nc.tensor.matmul(C1[:], W["c1big"][:], c0[:], start=True, stop=True)
                    c1h = actp.tile([128, 512], bf16, tag="c1h")
                    if j % 2 == 0:
                        nc.scalar.activation(c1h[:], C1[:], AF.Relu)
                    else:
                        nc.vector.tensor_scalar_max(c1h[:], C1[:], 0.0)

                    # C2 of pairs (2t, 2t+1) share a [128,1024] 2-bank tile;
                    # ONE fused evac after the odd pair.  Its consumers (C3/
                    # SIG) are deferred anyway, so the longer fused-evac
                    # latency is off the critical path.
                    if j % 2 == 0:
                        C2T = psc2p.tile([128, 1024], f32, tag="C2T")
                    half = slice(512 * (j % 2), 512 * (j % 2) + 512)
                    nc.tensor.matmul(C2T[:, half], W["c2big"][:], c1h[:],
                                     start=True, stop=True)
                    c2q.append((Widx, s // 2, h1))
                    if j % 2 == 1:
                        hc = actp.tile([128, 1024], bf16, tag="hc")
                        nc.vector.tensor_scalar_max(hc[:], C2T[:], 0.0)
                        staged = [(Wi, gi, hc[:, 512 * i: 512 * i + 512], h1i)
                                  for i, (Wi, gi, h1i) in enumerate(c2q)]
                        c2q = []

                    # emit one deferred C3/SIG per pair body
                    if defq:
                        emit_c3(*defq.pop(0))
            if staged is not None:
                defq.extend(staged)
            while defq:
                emit_c3(*defq.pop(0))
            if sig_pend is not None:
                sig_chain(*sig_pend, last=True)
    nc.compile()
    return nc


def _run(inputs, npts=NPC, trace=False, cores=N_CORES):
    from concourse import bass_utils

    key = npts
    if key not in _CACHE:
        _CACHE[key] = _build(npts)
    nc = _CACHE[key]
    wm = _pack_weights(inputs["ws0"], inputs["ws1"], inputs["ws2"],
                       inputs["wc0"], inputs["wc1"], inputs["wc2"], inputs["wc3"])
    t4 = _pack_input(inputs["x"])
    in_maps = [dict(wm, x8t=np.ascontiguousarray(t4[c])) for c in range(cores)]
    res = bass_utils.run_bass_kernel_spmd(
        nc, in_maps, core_ids=list(range(cores)), trace=trace)
    dev = np.stack([r["out"] for r in res.results], axis=0)
    dev2 = np.stack([r["out2"] for r in res.results], axis=0)
    return _unpack_output(dev, dev2), res


def kernel(**inputs):
    out, _ = _run(inputs)
    return out.astype(np.float32)


# revision 34
# speedup vs baseline: 1.0701x; 1.0684x over previous
"""NeRF-small MLP Bass kernel for Trainium2, 8-core data parallel.

v2 layout: hidden-on-partitions, points-on-free-dim, with HOST-side
input/output permutation so the device does zero transposes and zero
repacks.

Input: host pre-packs x into t4-layout [N_ST, 128, 512] bf16 where row
8q+c = channel c (c<6, pads zero) of slab q, col 128k+j = point
base + 2048k + 16j + q of the super-tile.  One contiguous HWDGE DMA per
super-tile.

Compute per pair of slabs j (1024 pts; every matmul K<=128, N=512):
  H0 = w0big_j x t4                     (3->64 both slabs)
  H1 = s1big x h0                       (64->64)
  CV = c0vbig_j x t4 + compbig x h1     (view path + composed geo path)
  C1 = c1big x c0
  C2 = c2big x c1h
  C3C += c3big_W x c2h ; += sigbig_W x h1   (final layer)

Output: one [128,512] f32 PSUM bank accumulates the final layer for 16
pairs (2 super-tiles): colors of pair W in rows 6W..6W+5, sigma_raw in
rows 96+2W..97+2W (other stationary columns are zero -> +0).  Evacuated
once per 2 super-tiles with ONE ACT Copy [128,512] into an SBUF tile
that is stored channel-major bf16 (colors + raw sigma).  Sigma then
takes a side path that keeps ACT/DVE free: a gpsimd SBUF-shuffle DMA
re-shapes the raw [32,512] sigma block to [128,128] so the softplus
polynomial (sigma = relu(x) + t*R(t), t = exp(-|x|), R deg-3) runs as
dense [128,128] ops: Abs/Exp on ACT, the 6-op Horner + relu + add on
the otherwise-idle Pool engine.  The bf16 result is stored as its own
small tensor; the host unpermutes everything to [N,4] f32.  No device
transposes, one activation table (Relu/Copy/Abs/Exp).

PSUM banks (8): H0 x2, H1, CV, C1, C2, C3C x2.
Evacuation engines statically balanced: ACT {h0, c0, c2h 5/8, output
Copy/Abs/Exp}, DVE {h1, c1h, c2h 3/8}.
"""

import numpy as np
import ml_dtypes

N_TOTAL = 1048576
N_CORES = 8
NPC = N_TOTAL // N_CORES       # 131072 points per core
ST = 8192                      # points per super-tile (4 chunks of 2048)
N_ST = NPC // ST               # 16
FP8_SCALE = 1024.0             # power-of-2, exact to un-scale

_CACHE = {}


def _pack_weights(ws0, ws1, ws2, wc0, wc1, wc2, wc3):
    """Build block-diagonal 'big' stationary matrices."""
    bf16 = ml_dtypes.bfloat16
    f32 = np.float32
    ws0, ws1, ws2, wc0, wc1, wc2, wc3 = [
        np.asarray(w, f32) for w in (ws0, ws1, ws2, wc0, wc1, wc2, wc3)
    ]
    w0big = np.zeros((128, 8 * 128), f32)
    c0vbig = np.zeros((128, 8 * 128), f32)
    for j in range(8):
        for q, off in ((2 * j, 0), (2 * j + 1, 64)):
            w0big[8 * q: 8 * q + 3, 128 * j + off: 128 * j + off + 64] = ws0
            c0vbig[8 * q + 3: 8 * q + 6, 128 * j + off: 128 * j + off + 64] = wc0[0:3]
    s1big = np.zeros((128, 128), f32)
    s1big[0:64, 0:64] = ws1
    s1big[64:128, 64:128] = ws1
    # geo path composed offline: geo @ wc0[3:18] = h1 @ (ws2[:,1:16] @ wc0[3:18])
    comp = ws2[:, 1:16] @ wc0[3:18]
    compbig = np.zeros((128, 128), f32)
    compbig[0:64, 0:64] = comp
    compbig[64:128, 64:128] = comp
    c1big = np.zeros((128, 128), f32)
    c1big[0:64, 0:64] = wc1
    c1big[64:128, 64:128] = wc1
    c2big = np.zeros((128, 128), f32)
    c2big[0:64, 0:64] = wc2
    c2big[64:128, 64:128] = wc2
    # Final layer: 16 pair-variants W, each a stationary writing the shared
    # [128,512] C3C accumulation bank.
    c3big = np.zeros((128, 16 * 128), f32)
    sigbig = np.zeros((128, 16 * 128), f32)
    for W in range(16):
        c3big[0:64, 128 * W + 6 * W: 128 * W + 6 * W + 3] = wc3[:, 0:3]
        c3big[64:128, 128 * W + 6 * W + 3: 128 * W + 6 * W + 6] = wc3[:, 0:3]
        sigbig[0:64, 128 * W + 96 + 2 * W] = ws2[:, 0]
        sigbig[64:128, 128 * W + 97 + 2 * W] = ws2[:, 0]

    return {
        "w0big": w0big.astype(bf16), "c0vbig": c0vbig.astype(bf16),
        "s1big": s1big.astype(bf16), "compbig": compbig.astype(bf16),
        "c1big": c1big.astype(bf16), "c2big": c2big.astype(bf16),
        "c3big": c3big.astype(bf16), "sigbig": sigbig.astype(bf16),
    }


def _pack_input(x):
    """[N,6] f32 -> [cores, N_ST, 128, 512] bf16 t4 layout:
    t4[8q+c, 128k+j] = x[base + 2048k + 16j + q, c], pads (c=6,7) zero."""
    bf16 = ml_dtypes.bfloat16
    x = np.asarray(x, np.float32).reshape(N_CORES, N_ST, 4, 128, 16, 6)
    t4 = np.zeros((N_CORES, N_ST, 16, 8, 4, 128), bf16)
    # [core, s, k, j, q, c] -> [core, s, q, c, k, j]
    t4[:, :, :, 0:6] = x.transpose(0, 1, 4, 5, 2, 3).astype(bf16)
    return t4.reshape(N_CORES, N_ST, 128, 512)


def _unpack_output(dev, dev2):
    """dev [cores, N_ST//2, 128, 512] (colors rows 0-95), dev2 [cores,
    N_ST//2, 128, 128] (softplus'd sigma, shuffled) -> [N, 4] f32.
    Pair W (0..15) within group g spans ST = 2g + W//8, slabs q = 2w, 2w+1
    (w = W%8); its column 128k+j is point (ST*4+k)*2048 + 16j + q.
    dev2[32k + 2W + par, j] = sigma of that point."""
    ns2 = N_ST // 2
    dev = dev.reshape(N_CORES, ns2, 128, 4, 128)
    # colors rows 6W+3*par+c
    colors = dev[:, :, :96].reshape(N_CORES, ns2, 2, 8, 2, 3, 4, 128)
    # -> [core, g, st2, k, j, w, par, c]
    colors = colors.transpose(0, 1, 2, 6, 7, 3, 4, 5)
    colors = np.ascontiguousarray(colors).reshape(N_TOTAL, 3).astype(np.float32)
    sigma = dev2.reshape(N_CORES, ns2, 4, 2, 8, 2, 128)  # [core,g,k,st2,w,par,j]
    sigma = sigma.transpose(0, 1, 3, 2, 6, 4, 5)         # [core,g,st2,k,j,w,par]
    sigma = np.ascontiguousarray(sigma).reshape(N_TOTAL, 1).astype(np.float32)
    return np.concatenate([colors, sigma], axis=1)


def _build(npts):
    import concourse.mybir as mybir
    from concourse import bacc, tile
    from concourse.bass import AP

    dt = mybir.dt
    f32, bf16 = dt.float32, dt.bfloat16
    AF = mybir.ActivationFunctionType

    ALU = mybir.AluOpType
    # ln(1+t)/t ~= B0 + B1 t + B2 t^2 + B3 t^3 on [0,1] (Chebyshev fit,
    # max abs err in t*R(t) is 5.1e-4)
    B0, B1, B2, B3 = 0.99930126, -0.48463524, 0.25187429, -0.0738988

    n_st = npts // ST
    nc = bacc.Bacc()
    x8t = nc.dram_tensor("x8t", [n_st, 128, 512], bf16, kind="ExternalInput")
    out = nc.dram_tensor("out", [n_st // 2, 128, 512], bf16, kind="ExternalOutput")
    out2 = nc.dram_tensor("out2", [n_st // 2, 128, 128], bf16, kind="ExternalOutput")
    wshapes = {
        "w0big": [128, 8 * 128], "c0vbig": [128, 8 * 128],
        "s1big": [128, 128], "compbig": [128, 128],
        "c1big": [128, 128], "c2big": [128, 128],
        "c3big": [128, 16 * 128], "sigbig": [128, 16 * 128],
    }
    wdr = {n: nc.dram_tensor(n, s, bf16, kind="ExternalInput")
           for n, s in wshapes.items()}

    with tile.TileContext(nc) as tc:
        with (
            tc.tile_pool(name="const", bufs=1) as constp,
            tc.tile_pool(name="t4", bufs=4) as t4p,
            tc.tile_pool(name="act", bufs=4) as actp,
            tc.tile_pool(name="ost", bufs=2) as ostp,
            tc.tile_pool(name="psum", bufs=1, space="PSUM") as psump,
            tc.tile_pool(name="psh0", bufs=2, space="PSUM") as psh0p,
            tc.tile_pool(name="psc3", bufs=2, space="PSUM") as psc3p,
        ):
            W = {}
            for name, shp in wshapes.items():
                t = constp.tile(shp, bf16, tag=name)
                nc.gpsimd.dma_start(t[:], wdr[name][:])
                W[name] = t

            def prefetch(s):
                t4 = t4p.tile([128, 512], bf16, tag="t4")
                with tc.high_priority():
                    nc.sync.dma_start(t4[:], x8t[s])
                return t4

            t4q = [prefetch(0), prefetch(1), prefetch(2)]

            C3C = None
            sig_pend = None   # (grp, sg) awaiting the softplus chain

            def emit_c3(Widx, grp, c2h, h1):
                nonlocal C3C, sig_pend
                if Widx == 0:
                    C3C = psc3p.tile([128, 512], f32, tag="C3C")
                nc.tensor.matmul(C3C[:], W["c3big"][:, 128 * Widx: 128 * (Widx + 1)],
                                 c2h, start=(Widx == 0), stop=False)
                nc.tensor.matmul(C3C[:], W["sigbig"][:, 128 * Widx: 128 * (Widx + 1)],
                                 h1[:], start=False, stop=(Widx == 15))
                if Widx == 15:
                    # one evac releases the bank: colors + raw sigma
                    ost = ostp.tile([128, 512], bf16, tag="ost")
                    nc.vector.tensor_copy(ost[:], C3C[:])
                    nc.sync.dma_start(out[grp], ost[:])
                    # shuffle raw sigma [32,512] -> [128,128] for the poly
                    sg = ostp.tile([128, 128], bf16, tag="sg")
                    for k in range(4):
                        nc.sync.dma_start(
                            sg[32 * k: 32 * (k + 1), :],
                            ost[96:128, 128 * k: 128 * (k + 1)])
                    sig_pend = (grp, sg)

            def sig_chain(grp, sg, last=False):
                # softplus(x) = relu(x) + t*R(t), t = exp(-|x|); dense
                # [128,128] tiles: Abs/Exp on ACT, the rest on Pool (steady
                # state) or DVE (final group, latency-critical tail)
                eng = nc.vector if last else nc.gpsimd
                spa = ostp.tile([128, 128], f32, tag="spa")
                nc.scalar.activation(spa[:], sg[:], AF.Abs)
                spt = ostp.tile([128, 128], f32, tag="spt")
                nc.scalar.activation(spt[:], spa[:], AF.Exp, scale=-1.0)
                t2 = ostp.tile([128, 128], f32, tag="t2")
                eng.tensor_tensor(t2[:], spt[:], spt[:], op=ALU.mult)
                pA = ostp.tile([128, 128], f32, tag="pA")
                eng.tensor_scalar(pA[:], spt[:], B1, B0, op0=ALU.mult, op1=ALU.add)
                pB = ostp.tile([128, 128], f32, tag="pB")
                eng.tensor_scalar(pB[:], spt[:], B3, B2, op0=ALU.mult, op1=ALU.add)
                pC = ostp.tile([128, 128], f32, tag="pC")
                eng.tensor_tensor(pC[:], pB[:], t2[:], op=ALU.mult)
                pD = ostp.tile([128, 128], f32, tag="pD")
                eng.tensor_tensor(pD[:], pA[:], pC[:], op=ALU.add)
                pP = ostp.tile([128, 128], f32, tag="pP")
                eng.tensor_tensor(pP[:], pD[:], spt[:], op=ALU.mult)
                pr = ostp.tile([128, 128], f32, tag="pr")
                eng.tensor_scalar_max(pr[:], sg[:], 0.0)
                sg2 = ostp.tile([128, 128], bf16, tag="sg2")
                eng.tensor_tensor(sg2[:], pP[:], pr[:], op=ALU.add)
                nc.sync.dma_start(out2[grp], sg2[:])

            deferred = None
            for s in range(n_st):
                t4 = t4q.pop(0)
                if s + 3 < n_st:
                    t4q.append(prefetch(s + 3))

                for j in range(8):
                    Widx = 8 * (s % 2) + j
                    if j == 2 and sig_pend is not None:
                        sig_chain(*sig_pend)
                        sig_pend = None
                    H0 = psh0p.tile([128, 512], f32, tag="H0")
                    nc.tensor.matmul(H0[:], W["w0big"][:, 128 * j: 128 * (j + 1)],
                                     t4[:], start=True, stop=True)
                    h0 = actp.tile([128, 512], bf16, tag="h0")
                    nc.scalar.activation(h0[:], H0[:], AF.Relu)

                    H1 = psump.tile([128, 512], f32, tag="H1")
                    nc.tensor.matmul(H1[:], W["s1big"][:], h0[:], start=True, stop=True)
                    h1 = actp.tile([128, 512], bf16, tag="h1")
                    nc.vector.tensor_scalar_max(h1[:], H1[:], 0.0)

                    CV = psump.tile([128, 512], f32, tag="CV")
                    nc.tensor.matmul(CV[:], W["c0vbig"][:, 128 * j: 128 * (j + 1)],
                                     t4[:], start=True, stop=False)
                    nc.tensor.matmul(CV[:], W["compbig"][:], h1[:], start=False, stop=True)
                    c0 = actp.tile([128, 512], bf16, tag="c0")
                    nc.scalar.activation(c0[:], CV[:], AF.Relu)

                    C1 = psump.tile([128, 512], f32, tag="C1")
                    nc.tensor.matmul(C1[:], W["c1big"][:], c0[:], start=True, stop=True)
                    c1h = actp.tile([128, 512], bf16, tag="c1h")
                    nc.vector.tensor_scalar_max(c1h[:], C1[:], 0.0)

                    C2 = psump.tile([128, 512], f32, tag="C2")
                    nc.tensor.matmul(C2[:], W["c2big"][:], c1h[:], start=True, stop=True)
                    c2h = actp.tile([128, 512], bf16, tag="c2h")
                    if j % 8 < 4:
                        nc.scalar.activation(c2h[:], C2[:], AF.Relu)
                    else:
                        nc.vector.tensor_scalar_max(c2h[:], C2[:], 0.0)

                    # C3/SIG for pair Widx are emitted during the NEXT pair so
                    # the PE never reaches a matmul whose mover was produced by
                    # the immediately preceding matmul's evacuation.
                    if deferred is not None:
                        emit_c3(*deferred)
                    deferred = (Widx, s // 2, c2h[:], h1)
            emit_c3(*deferred)
            if sig_pend is not None:
                sig_chain(*sig_pend, last=True)
    nc.compile()
    return nc


def _run(inputs, npts=NPC, trace=False, cores=N_CORES):
    from concourse import bass_utils

    key = npts
    if key not in _CACHE:
        _CACHE[key] = _build(npts)
    nc = _CACHE[key]
    wm = _pack_weights(inputs["ws0"], inputs["ws1"], inputs["ws2"],
                       inputs["wc0"], inputs["wc1"], inputs["wc2"], inputs["wc3"])
    t4 = _pack_input(inputs["x"])
    in_maps = [dict(wm, x8t=np.ascontiguousarray(t4[c])) for c in range(cores)]
    res = bass_utils.run_bass_kernel_spmd(
        nc, in_maps, core_ids=list(range(cores)), trace=trace)
    dev = np.stack([r["out"] for r in res.results], axis=0)
    dev2 = np.stack([r["out2"] for r in res.results], axis=0)
    return _unpack_output(dev, dev2), res


def kernel(**inputs):
    out, _ = _run(inputs)
    return out.astype(np.float32)


# revision 35
# speedup vs baseline: 1.1316x; 1.0574x over previous
"""NeRF-small MLP Bass kernel for Trainium2, 8-core data parallel.

v2 layout: hidden-on-partitions, points-on-free-dim, with HOST-side
input/output permutation so the device does zero transposes and zero
repacks.

Input: host pre-packs x into t4-layout [N_ST, 128, 512] bf16 where row
8q+c = channel c (c<6, pads zero) of slab q, col 128k+j = point
base + 2048k + 16j + q of the super-tile.  One contiguous HWDGE DMA per
super-tile.

Compute per pair of slabs j (1024 pts; every matmul K<=128, N=512):
  H0 = w0big_j x t4                     (3->64 both slabs)
  H1 = s1big x h0                       (64->64)
  CV = c0vbig_j x t4 + compbig x h1     (view path + composed geo path)
  C1 = c1big x c0
  C2 = c2big x c1h
  C3C += c3big_W x c2h ; += sigbig_W x h1   (final layer)

Output: one [128,512] f32 PSUM bank accumulates the final layer for 16
pairs (2 super-tiles): colors of pair W in rows 6W..6W+5, sigma_raw in
rows 96+2W..97+2W (other stationary columns are zero -> +0).  Evacuated
once per 2 super-tiles with ONE ACT Copy [128,512] into an SBUF tile
that is stored channel-major bf16 (colors + raw sigma).  Sigma then
takes a side path that keeps ACT/DVE free: a gpsimd SBUF-shuffle DMA
re-shapes the raw [32,512] sigma block to [128,128] so the softplus
polynomial (sigma = relu(x) + t*R(t), t = exp(-|x|), R deg-3) runs as
dense [128,128] ops: Abs/Exp on ACT, the 6-op Horner + relu + add on
the otherwise-idle Pool engine.  The bf16 result is stored as its own
small tensor; the host unpermutes everything to [N,4] f32.  No device
transposes, one activation table (Relu/Copy/Abs/Exp).

PSUM banks (8): H0 x2, H1, CV, C1, C2, C3C x2.
Evacuation engines statically balanced: ACT {h0, c0, c2h 5/8, output
Copy/Abs/Exp}, DVE {h1, c1h, c2h 3/8}.
"""

import numpy as np
import ml_dtypes

N_TOTAL = 1048576
N_CORES = 8
NPC = N_TOTAL // N_CORES       # 131072 points per core
ST = 8192                      # points per super-tile (4 chunks of 2048)
N_ST = NPC // ST               # 16
FP8_SCALE = 1024.0             # power-of-2, exact to un-scale

_CACHE = {}


def _pack_weights(ws0, ws1, ws2, wc0, wc1, wc2, wc3):
    """Build block-diagonal 'big' stationary matrices."""
    bf16 = ml_dtypes.bfloat16
    f32 = np.float32
    ws0, ws1, ws2, wc0, wc1, wc2, wc3 = [
        np.asarray(w, f32) for w in (ws0, ws1, ws2, wc0, wc1, wc2, wc3)
    ]
    w0big = np.zeros((128, 8 * 128), f32)
    c0vbig = np.zeros((128, 8 * 128), f32)
    for j in range(8):
        for q, off in ((2 * j, 0), (2 * j + 1, 64)):
            w0big[8 * q: 8 * q + 3, 128 * j + off: 128 * j + off + 64] = ws0
            c0vbig[8 * q + 3: 8 * q + 6, 128 * j + off: 128 * j + off + 64] = wc0[0:3]
    s1big = np.zeros((128, 128), f32)
    s1big[0:64, 0:64] = ws1
    s1big[64:128, 64:128] = ws1
    # geo path composed offline: geo @ wc0[3:18] = h1 @ (ws2[:,1:16] @ wc0[3:18])
    comp = ws2[:, 1:16] @ wc0[3:18]
    compbig = np.zeros((128, 128), f32)
    compbig[0:64, 0:64] = comp
    compbig[64:128, 64:128] = comp
    c1big = np.zeros((128, 128), f32)
    c1big[0:64, 0:64] = wc1
    c1big[64:128, 64:128] = wc1
    c2big = np.zeros((128, 128), f32)
    c2big[0:64, 0:64] = wc2
    c2big[64:128, 64:128] = wc2
    # Final layer: 16 pair-variants W, each a stationary writing the shared
    # [128,512] C3C accumulation bank.
    c3big = np.zeros((128, 16 * 128), f32)
    sigbig = np.zeros((128, 16 * 128), f32)
    for W in range(16):
        c3big[0:64, 128 * W + 6 * W: 128 * W + 6 * W + 3] = wc3[:, 0:3]
        c3big[64:128, 128 * W + 6 * W + 3: 128 * W + 6 * W + 6] = wc3[:, 0:3]
        sigbig[0:64, 128 * W + 96 + 2 * W] = ws2[:, 0]
        sigbig[64:128, 128 * W + 97 + 2 * W] = ws2[:, 0]

    return {
        "w0big": w0big.astype(bf16), "c0vbig": c0vbig.astype(bf16),
        "s1big": s1big.astype(bf16), "compbig": compbig.astype(bf16),
        "c1big": c1big.astype(bf16), "c2big": c2big.astype(bf16),
        "c3big": c3big.astype(bf16), "sigbig": sigbig.astype(bf16),
    }


def _pack_input(x):
    """[N,6] f32 -> [cores, N_ST, 128, 512] bf16 t4 layout:
    t4[8q+c, 128k+j] = x[base + 2048k + 16j + q, c], pads (c=6,7) zero."""
    bf16 = ml_dtypes.bfloat16
    x = np.asarray(x, np.float32).reshape(N_CORES, N_ST, 4, 128, 16, 6)
    t4 = np.zeros((N_CORES, N_ST, 16, 8, 4, 128), bf16)
    # [core, s, k, j, q, c] -> [core, s, q, c, k, j]
    t4[:, :, :, 0:6] = x.transpose(0, 1, 4, 5, 2, 3).astype(bf16)
    return t4.reshape(N_CORES, N_ST, 128, 512)


def _unpack_output(dev, dev2):
    """dev [cores, N_ST//2, 128, 512] (colors rows 0-95), dev2 [cores,
    N_ST//2, 128, 128] (softplus'd sigma, shuffled) -> [N, 4] f32.
    Pair W (0..15) within group g spans ST = 2g + W//8, slabs q = 2w, 2w+1
    (w = W%8); its column 128k+j is point (ST*4+k)*2048 + 16j + q.
    dev2[32k + 2W + par, j] = sigma of that point."""
    ns2 = N_ST // 2
    dev = dev.reshape(N_CORES, ns2, 128, 4, 128)
    # colors rows 6W+3*par+c
    colors = dev[:, :, :96].reshape(N_CORES, ns2, 2, 8, 2, 3, 4, 128)
    # -> [core, g, st2, k, j, w, par, c]
    colors = colors.transpose(0, 1, 2, 6, 7, 3, 4, 5)
    colors = np.ascontiguousarray(colors).reshape(N_TOTAL, 3).astype(np.float32)
    sigma = dev2.reshape(N_CORES, ns2, 4, 2, 8, 2, 128)  # [core,g,k,st2,w,par,j]
    sigma = sigma.transpose(0, 1, 3, 2, 6, 4, 5)         # [core,g,st2,k,j,w,par]
    sigma = np.ascontiguousarray(sigma).reshape(N_TOTAL, 1).astype(np.float32)
    return np.concatenate([colors, sigma], axis=1)


def _build(npts):
    import concourse.mybir as mybir
    from concourse import bacc, tile
    from concourse.bass import AP

    dt = mybir.dt
    f32, bf16 = dt.float32, dt.bfloat16
    AF = mybir.ActivationFunctionType

    ALU = mybir.AluOpType
    # ln(1+t)/t ~= B0 + B1 t + B2 t^2 + B3 t^3 on [0,1] (Chebyshev fit,
    # max abs err in t*R(t) is 5.1e-4)
    B0, B1, B2, B3 = 0.99930126, -0.48463524, 0.25187429, -0.0738988

    n_st = npts // ST
    nc = bacc.Bacc()
    x8t = nc.dram_tensor("x8t", [n_st, 128, 512], bf16, kind="ExternalInput")
    out = nc.dram_tensor("out", [n_st // 2, 128, 512], bf16, kind="ExternalOutput")
    out2 = nc.dram_tensor("out2", [n_st // 2, 128, 128], bf16, kind="ExternalOutput")
    wshapes = {
        "w0big": [128, 8 * 128], "c0vbig": [128, 8 * 128],
        "s1big": [128, 128], "compbig": [128, 128],
        "c1big": [128, 128], "c2big": [128, 128],
        "c3big": [128, 16 * 128], "sigbig": [128, 16 * 128],
    }
    wdr = {n: nc.dram_tensor(n, s, bf16, kind="ExternalInput")
           for n, s in wshapes.items()}

    with tile.TileContext(nc) as tc:
        with (
            tc.tile_pool(name="const", bufs=1) as constp,
            tc.tile_pool(name="t4", bufs=4) as t4p,
            tc.tile_pool(name="act", bufs=6) as actp,
            tc.tile_pool(name="ost", bufs=2) as ostp,
            tc.tile_pool(name="psum", bufs=1, space="PSUM") as psump,
            tc.tile_pool(name="psh0", bufs=2, space="PSUM") as psh0p,
            tc.tile_pool(name="psc3", bufs=2, space="PSUM") as psc3p,
        ):
            W = {}
            for name, shp in wshapes.items():
                t = constp.tile(shp, bf16, tag=name)
                nc.gpsimd.dma_start(t[:], wdr[name][:])
                W[name] = t

            def prefetch(s):
                t4 = t4p.tile([128, 512], bf16, tag="t4")
                with tc.high_priority():
                    nc.sync.dma_start(t4[:], x8t[s])
                return t4

            t4q = [prefetch(0), prefetch(1), prefetch(2)]

            C3C = None
            sig_pend = None   # (grp, sg) awaiting the softplus chain

            def emit_c3(Widx, grp, c2h, h1):
                nonlocal C3C, sig_pend
                if Widx == 0:
                    C3C = psc3p.tile([128, 512], f32, tag="C3C")
                nc.tensor.matmul(C3C[:], W["c3big"][:, 128 * Widx: 128 * (Widx + 1)],
                                 c2h, start=(Widx == 0), stop=False)
                nc.tensor.matmul(C3C[:], W["sigbig"][:, 128 * Widx: 128 * (Widx + 1)],
                                 h1[:], start=False, stop=(Widx == 15))
                if Widx == 15:
                    # one evac releases the bank: colors + raw sigma
                    ost = ostp.tile([128, 512], bf16, tag="ost")
                    nc.scalar.activation(ost[:], C3C[:], AF.Copy)
                    nc.gpsimd.dma_start(out[grp], ost[:])
                    # shuffle raw sigma [32,512] -> [128,128] for the poly
                    sg = ostp.tile([128, 128], bf16, tag="sg")
                    for k in range(4):
                        nc.sync.dma_start(
                            sg[32 * k: 32 * (k + 1), :],
                            ost[96:128, 128 * k: 128 * (k + 1)])
                    sig_pend = (grp, sg)

            def sig_chain(grp, sg, last=False):
                # softplus(x) = relu(x) + t*R(t), t = exp(-|x|); dense
                # [128,128] tiles: Abs/Exp on ACT, the rest on Pool (steady
                # state) or DVE (final group, latency-critical tail)
                eng = nc.vector if last else nc.gpsimd
                spa = ostp.tile([128, 128], f32, tag="spa")
                nc.scalar.activation(spa[:], sg[:], AF.Abs)
                spt = ostp.tile([128, 128], f32, tag="spt")
                nc.scalar.activation(spt[:], spa[:], AF.Exp, scale=-1.0)
                t2 = ostp.tile([128, 128], f32, tag="t2")
                eng.tensor_tensor(t2[:], spt[:], spt[:], op=ALU.mult)
                pA = ostp.tile([128, 128], f32, tag="pA")
                eng.tensor_scalar(pA[:], spt[:], B1, B0, op0=ALU.mult, op1=ALU.add)
                pB = ostp.tile([128, 128], f32, tag="pB")
                eng.tensor_scalar(pB[:], spt[:], B3, B2, op0=ALU.mult, op1=ALU.add)
                pC = ostp.tile([128, 128], f32, tag="pC")
                eng.tensor_tensor(pC[:], pB[:], t2[:], op=ALU.mult)
                pD = ostp.tile([128, 128], f32, tag="pD")
                eng.tensor_tensor(pD[:], pA[:], pC[:], op=ALU.add)
                pP = ostp.tile([128, 128], f32, tag="pP")
                eng.tensor_tensor(pP[:], pD[:], spt[:], op=ALU.mult)
                pr = ostp.tile([128, 128], f32, tag="pr")
                eng.tensor_scalar_max(pr[:], sg[:], 0.0)
                sg2 = ostp.tile([128, 128], bf16, tag="sg2")
                eng.tensor_tensor(sg2[:], pP[:], pr[:], op=ALU.add)
                nc.sync.dma_start(out2[grp], sg2[:])

            deferred = None
            for s in range(n_st):
                t4 = t4q.pop(0)
                if s + 3 < n_st:
                    t4q.append(prefetch(s + 3))

                for j in range(8):
                    Widx = 8 * (s % 2) + j
                    if j == 2 and sig_pend is not None:
                        sig_chain(*sig_pend)
                        sig_pend = None
                    H0 = psh0p.tile([128, 512], f32, tag="H0")
                    nc.tensor.matmul(H0[:], W["w0big"][:, 128 * j: 128 * (j + 1)],
                                     t4[:], start=True, stop=True)
                    h0 = actp.tile([128, 512], bf16, tag="h0")
                    nc.scalar.activation(h0[:], H0[:], AF.Relu)

                    H1 = psump.tile([128, 512], f32, tag="H1")
                    nc.tensor.matmul(H1[:], W["s1big"][:], h0[:], start=True, stop=True)
                    h1 = actp.tile([128, 512], bf16, tag="h1")
                    nc.vector.tensor_scalar_max(h1[:], H1[:], 0.0)

                    CV = psump.tile([128, 512], f32, tag="CV")
                    nc.tensor.matmul(CV[:], W["c0vbig"][:, 128 * j: 128 * (j + 1)],
                                     t4[:], start=True, stop=False)
                    nc.tensor.matmul(CV[:], W["compbig"][:], h1[:], start=False, stop=True)
                    c0 = actp.tile([128, 512], bf16, tag="c0")
                    nc.scalar.activation(c0[:], CV[:], AF.Relu)

                    C1 = psump.tile([128, 512], f32, tag="C1")
                    nc.tensor.matmul(C1[:], W["c1big"][:], c0[:], start=True, stop=True)
                    c1h = actp.tile([128, 512], bf16, tag="c1h")
                    nc.vector.tensor_scalar_max(c1h[:], C1[:], 0.0)

                    C2 = psump.tile([128, 512], f32, tag="C2")
                    nc.tensor.matmul(C2[:], W["c2big"][:], c1h[:], start=True, stop=True)
                    c2h = actp.tile([128, 512], bf16, tag="c2h")
                    if j % 8 < 4:
                        nc.scalar.activation(c2h[:], C2[:], AF.Relu)
                    else:
                        nc.vector.tensor_scalar_max(c2h[:], C2[:], 0.0)

                    # C3/SIG for pair Widx are emitted during the NEXT pair so
                    # the PE never reaches a matmul whose mover was produced by
                    # the immediately preceding matmul's evacuation.
                    if deferred is not None:
                        emit_c3(*deferred)
                    deferred = (Widx, s // 2, c2h[:], h1)
            emit_c3(*deferred)
            if sig_pend is not None:
                sig_chain(*sig_pend, last=True)
    nc.compile()
    return nc


def _run(inputs, npts=NPC, trace=False, cores=N_CORES):
    from concourse import bass_utils

    key = npts
    if key not in _CACHE:
        _CACHE[key] = _build(npts)
    nc = _CACHE[key]
    wm = _pack_weights(inputs["ws0"], inputs["ws1"], inputs["ws2"],
                       inputs["wc0"], inputs["wc1"], inputs["wc2"], inputs["wc3"])
    t4 = _pack_input(inputs["x"])
    in_maps = [dict(wm, x8t=np.ascontiguousarray(t4[c])) for c in range(cores)]
    res = bass_utils.run_bass_kernel_spmd(
        nc, in_maps, core_ids=list(range(cores)), trace=trace)
    dev = np.stack([r["out"] for r in res.results], axis=0)
    dev2 = np.stack([r["out2"] for r in res.results], axis=0)
    return _unpack_output(dev, dev2), res


def kernel(**inputs):
    out, _ = _run(inputs)
    return out.astype(np.float32)
